# revision 52
# baseline (speedup 1.0000x reference)
"""Multi-head attention (B=2, S=4096, D=768, H=12) on 8 TRN2 NeuronCores.

Sharding: 24 (batch, head) pairs -> 3 heads per core. Cores 0-3 take batch 0,
cores 4-7 take batch 1. Each core computes q/k/v projections for its 3 heads,
flash-style attention (scores kept transposed [kv, q] so exp can run straight
out of PSUM), and a partial output projection over its 192 contraction rows.
The host sums the 4 partial outputs per batch and adds the output bias.

v3 structure (456 -> 430 us lineage):
- q/k/v projections run in fp8 DoubleRow (contraction 768 = 3 pairs of
  128-chunks), halving projection matmul slots; qTx/kTx/vTx ship as fp8.
- Inputs land as two half-tile DMAs per 512-seq block on the sync/scalar
  HWDGE queues (gpsimd DGE is software-paced, ~1us/transfer - only used
  for bulk v-tiles). First k-tile is split per-kc so matmuls start early.
- Score tiles sit in a 3-deep [128,1024] PSUM ring (the third slot is
  carved from the old q-proj accumulator pool; q-proj accumulators now
  borrow score-ring slots for their 4-step lifetime). Ring-3 covers the
  exp dependency chain (MM ~385ns + sem + exp ~1.2us > 2-slot window),
  which was the main tensor-stall source at ring-2.
- Softmax exp alternates engines per kv pair: ACT true exp (fp8 out),
  DVE Schraudolph u8 = round(s*A + B) bit-cast as fp8e4m3. q-proj drains
  moved to ACT (Identity activation with per-partition bias rides the exp
  table set) to keep DVE exp cadence ahead of the score ring.
- P@V runs in fp8 DoubleRow: each matmul contracts TWO kv chunks. The
  denominator falls out via a ones-column appended to V; its partition
  broadcast takes a DRAM round trip, reciprocal+scale deferred one head.
- Output projection of q-chunk qc-1 interleaves into heads 0/1 of qc.
"""

import sys

sys.path.insert(0, "/opt/trn_rl_repo")

import numpy as np  # noqa: E402

from concourse import bacc, bass, mybir, tile  # noqa: E402
from concourse.bass_utils import run_bass_kernel_spmd  # noqa: E402


S = 4096
DM = 768
DK = 64
HPC = 3  # heads per core
NC_CORES = 8
KC = DM // 128  # 6 contraction chunks for projections
NSB = S // 512  # 8 seq blocks (projection N / attention q chunks)
NKV = S // 128  # 32 kv chunks
SCALE = 1.0 / np.sqrt(DK)
# Schraudolph fp8e4m3 exp: u8 = round(s * A + B), bits viewed as fp8.
A_SCH = float(8.0 * np.log2(np.e) * SCALE)
B_SCH = float(56.0 - 0.3443)
DVE_PAIRS = (1, 3, 5, 7, 9, 11, 13, 15)  # exp pairs computed on the vector engine

F16 = mybir.dt.float16
F32 = mybir.dt.float32
F8 = mybir.dt.float8e4
U8 = mybir.dt.uint8
DR = mybir.MatmulPerfMode.DoubleRow


def _emit(tc):
    nc = tc.nc
    qTx = nc.dram_tensor("qTx", [NSB, 128, KC, 512], F8, kind="ExternalInput").ap()
    kTx = nc.dram_tensor("kTx", [NSB, 128, KC, 512], F8, kind="ExternalInput").ap()
    vTx = nc.dram_tensor("vTx", [NSB, 128, KC, 512], F8, kind="ExternalInput").ap()
    wqT = nc.dram_tensor("wqT", [DM, HPC * DK], F8, kind="ExternalInput").ap()
    wkT = nc.dram_tensor("wkT", [DM, HPC * DK], F8, kind="ExternalInput").ap()
    wvT = nc.dram_tensor("wvT", [DM, HPC * DK], F8, kind="ExternalInput").ap()
    woT = nc.dram_tensor("woT", [HPC * DK, DM], F16, kind="ExternalInput").ap()
    bq = nc.dram_tensor("bq", [HPC * DK, 1], F32, kind="ExternalInput").ap()
    bk = nc.dram_tensor("bk", [HPC * DK, 1], F32, kind="ExternalInput").ap()
    bv = nc.dram_tensor("bv", [HPC * DK, 1], F32, kind="ExternalInput").ap()
    out_pT = nc.dram_tensor("out_pT", [DM, S], F16, kind="ExternalOutput").ap()
    den_d = nc.dram_tensor("den_d", [NSB * HPC, 512], F32, kind="Internal").ap()

    with (
        tc.tile_pool(name="const", bufs=1) as const,
        tc.tile_pool(name="heads", bufs=1) as heads,
        tc.tile_pool(name="xts", bufs=8) as xts,
        tc.tile_pool(name="work", bufs=3) as work,
        tc.tile_pool(name="norm", bufs=4) as norm,
    ):
        # ---- constants -------------------------------------------------
        w_q = const.tile([128, KC, HPC * DK], F8, tag="w_q")
        w_k = const.tile([128, KC, HPC * DK], F8, tag="w_k")
        w_v = const.tile([128, KC, HPC * DK], F8, tag="w_v")
        # constants ride the (otherwise idle) gpsimd DGE queue so they never
        # queue behind the 768 KiB x-tile streams on sync/scalar.
        nc.sync.dma_start(w_k[:], wkT.rearrange("(c p) m -> p c m", p=128))
        wo01 = const.tile([128, DM], F16, tag="wo01")
        wo2 = const.tile([DK, DM], F16, tag="wo2")
        bq01 = const.tile([128, 1], F32, tag="bq01")
        bq2 = const.tile([DK, 1], F32, tag="bq2")
        bk01 = const.tile([128, 1], F32, tag="bk01")
        bk2 = const.tile([DK, 1], F32, tag="bk2")
        nc.sync.dma_start(bk01[:], bk[0:128, :])
        nc.sync.dma_start(bk2[:], bk[128:192, :])
        # v-bias broadcast to all 128 partitions: bvb[p, j] = bv[j]
        bvb = const.tile([128, HPC * DK], F32, tag="bvb")
        bv_bcast = bass.AP(
            tensor=bv.tensor, offset=bv.offset, ap=[[0, 128]] + list(bv.ap)
        )

        # preload the exp activation table during the projection phase
        warm = const.tile([1, 1], F32, tag="warm2")
        nc.vector.memset(warm[:], 0.0)
        nc.scalar.activation(warm[:], warm[:], mybir.ActivationFunctionType.Exp)

        # ---- per-head persistent tensors ------------------------------
        # qT2/kT2: [128, S] fp16, rows 0:64 and 64:128 both hold head's
        # qT/kT (duplicated so row-tiled matmul pairs can stream from
        # either partition half).
        qT2 = [heads.tile([128, S], F16, tag=f"qT2_{h}", name=f"qT2_{h}") for h in range(HPC)]
        kT2 = [heads.tile([128, S], F16, tag=f"kT2_{h}", name=f"kT2_{h}") for h in range(HPC)]
        # v_dr: [128, NKV, 80] fp8; [p, g, 0:64] = v of kv chunk g (seq pos p
        # on partitions), col 64 = 1.0 (denominator column), 65:80 pad.
        v_dr = [heads.tile([128, NKV, 80], F8, tag=f"v_dr_{h}", name=f"v_dr_{h}") for h in range(HPC)]
        for h in range(HPC):
            nc.vector.memset(v_dr[h][:], 1.0)
        # normalized context, transposed: ctx01 rows 0:64 = head 0, rows
        # 64:128 = head 1; ctx2 = head 2. Together the lhsT of the output
        # projection.
        ctx01 = heads.tile([128, S], F16, tag="ctx01")
        ctx2 = heads.tile([64, S], F16, tag="ctx2")

        def dma_in(tile_ap, src, i, fine=False):
            # split each 768 KiB x-tile into two half-transfers on the two
            # HWDGE issue engines: first data lands in half the time and the
            # two queues stay balanced. fine=True splits per-kc so the first
            # matmul can start after a single 128 KiB transfer.
            if fine:
                for kc in range(KC):
                    eng = nc.sync if (i + kc) % 2 == 0 else nc.scalar
                    eng.dma_start(tile_ap[:, kc, :], src[:, kc, :])
                return
            h = KC // 2
            eng0, eng1 = (nc.sync, nc.scalar) if i % 2 == 0 else (nc.scalar, nc.sync)
            eng0.dma_start(tile_ap[:, 0:h, :], src[:, 0:h, :])
            eng1.dma_start(tile_ap[:, h:KC, :], src[:, h:KC, :])

        # ---- projections: k --------------------------------------------
        with tc.tile_pool(name="pp", bufs=2, space=bass.MemorySpace.PSUM) as pp:
            kxs = []
            for sb in range(NSB):
                kx = xts.tile([128, KC, 512], F8, tag="x8", bufs=10, name=f"kx_{sb}")
                dma_in(kx[:], kTx[sb], sb, fine=(sb == 0))
                kxs.append(kx)
            for sb in range(NSB):
                sq = bass.ts(sb, 512)
                k01 = pp.tile([128, 512], F32, tag="k01")
                k2 = pp.tile([DK, 512], F32, tag="k2")
                for c in range(KC // 2):
                    st = dict(start=(c == 0), stop=(c == KC // 2 - 1))
                    xsl = kxs[sb][:, 2 * c : 2 * c + 2, :]
                    nc.tensor.matmul(k01[:], w_k[:, 2 * c : 2 * c + 2, 0:128],
                                     xsl, perf_mode=DR, **st)
                    nc.tensor.matmul(k2[:], w_k[:, 2 * c : 2 * c + 2, 128:192],
                                     xsl, perf_mode=DR, **st)
                nc.vector.tensor_scalar_add(kT2[0][0:64, sq], k01[0:64, :], bk01[0:64, :])
                nc.vector.tensor_scalar_add(kT2[1][0:64, sq], k01[64:128, :], bk01[64:128, :])
                nc.vector.tensor_scalar_add(kT2[2][0:64, sq], k2[:], bk2[:])
                for h in range(HPC):
                    nc.vector.tensor_copy(out=kT2[h][64:128, sq], in_=kT2[h][0:64, sq])

        # v-proj inputs early so the DMA stream stays busy
        nc.sync.dma_start(w_v[:], wvT.rearrange("(c p) m -> p c m", p=128))
        nc.sync.dma_start(bvb[:], bv_bcast)

        # ---- projections: v -------------------------------------------
        # v rows (seq) on partitions: out tile [128 seq, 192] per kv chunk.
        with tc.tile_pool(name="vp", bufs=4, space=bass.MemorySpace.PSUM) as vp:
            vxs = []
            for sb in range(NSB):
                vx = xts.tile([128, KC, 512], F8, tag="x8", bufs=10, name=f"vx_{sb}")
                dma_in(vx[:], vTx[sb], sb)
                vxs.append(vx)
            for sb in range(NSB):
                for ss in range(4):  # kv chunk index = 4*sb + ss
                    vps = vp.tile([128, HPC * DK], F32, tag="vps")
                    for c in range(KC // 2):
                        nc.tensor.matmul(
                            vps[:],
                            vxs[sb][:, 2 * c : 2 * c + 2, bass.ds(ss * 128, 128)],
                            w_v[:, 2 * c : 2 * c + 2, :],
                            start=(c == 0),
                            stop=(c == KC // 2 - 1),
                            perf_mode=DR,
                        )
                    g = 4 * sb + ss
                    for h in range(HPC):
                        nc.vector.tensor_add(
                            v_dr[h][:, g, 0:64],
                            vps[:, bass.ts(h, 64)],
                            bvb[:, bass.ts(h, 64)],
                        )

        # remaining constant loads
        nc.sync.dma_start(w_q[:], wqT.rearrange("(c p) m -> p c m", p=128))
        nc.sync.dma_start(bq01[:], bq[0:128, :])
        nc.sync.dma_start(bq2[:], bq[128:192, :])
        nc.sync.dma_start(wo01[:], woT[0:128, :])
        nc.sync.dma_start(wo2[:], woT[128:192, :])

        # ---- attention + output projection ----------------------------
        # q chunks outer, heads inner. Scores land in [128, 1024] PSUM tiles
        # (one kv-chunk pair per exp, 3-deep ring shared with the q-proj
        # accumulator); P is written as fp8 into flat [128, 1024] staging
        # tiles, consumed by one DoubleRow P@V matmul per pair, issued two
        # slots late so its exp is long finished. The next q chunk's
        # projection rides inside head 2's loop; the previous q chunk's
        # output projection inside head 0's. Each head's normalize
        # (reciprocal + scale) is deferred into the NEXT head's loop so the
        # denominator's DRAM-broadcast round trip never blocks the DVE queue.
        with (
            tc.tile_pool(name="sp", bufs=3, space=bass.MemorySpace.PSUM) as sp,
            tc.tile_pool(name="bigp", bufs=2, space=bass.MemorySpace.PSUM) as bigp,
        ):
            def op_chain(qc, d, pool=None):
                # one sixth of q-chunk qc's output projection, wo stationary:
                # out^T[dm chunk d, qc seq block] = wo01_d.T@ctx01 + wo2_d.T@ctx2
                sq_ = bass.ts(qc, 512)
                dsl = bass.ts(d, 128)
                if pool is None:
                    op = bigp.tile([128, 512], F32, tag="big",
                                   name=f"op_{qc}_{d}")
                else:
                    opt = pool.tile([128, 1024], F32, tag="sT",
                                    name=f"opt_{qc}_{d}")
                    op = opt[:, 0:512]
                nc.tensor.matmul(op[:], wo01[:, dsl], ctx01[:, sq_],
                                 start=True, stop=False)
                nc.tensor.matmul(op[:], wo2[:, dsl], ctx2[:, sq_],
                                 start=False, stop=True)
                ob = work.tile([128, 512], F16, tag="ob", name=f"ob_{qc}_{d}")
                # negative offset = appear LATER to the scheduler, so the
                # next exps win the ACT queue position over this copy
                with tc.high_priority(offset=-25):
                    nc.scalar.copy(ob[:], op[:])
                    nc.sync.dma_start(out_pT[dsl, sq_], ob[:])

            qp_state = {}

            def qproj_step(qc, kc):
                if kc == 0:
                    qp_state[qc] = sp.tile([128, 1024], F32, tag="sT",
                                           name=f"qp_{qc}")
                    qx = xts.tile([128, KC, 512], F16, tag="xx",
                                  name=f"qx_{qc}")
                    dma_in(qx[:], qTx[qc], qc)
                    qp_state[f"x{qc}"] = qx
                qp = qp_state[qc]
                qx = qp_state[f"x{qc}"]
                st = dict(start=(kc == 0), stop=(kc == KC // 2 - 1))
                xsl = qx[:, 2 * kc : 2 * kc + 2, :]
                nc.tensor.matmul(qp[:, 0:512], w_q[:, 2 * kc : 2 * kc + 2, 0:128],
                                 xsl, perf_mode=DR, **st)
                nc.tensor.matmul(qp[0:64, 512:1024],
                                 w_q[:, 2 * kc : 2 * kc + 2, 128:192],
                                 xsl, perf_mode=DR, **st)

            def qproj_drain(qc):
                sq = bass.ts(qc, 512)
                qp = qp_state.pop(qc)
                qp_state.pop(f"x{qc}")
                nc.scalar.activation(qT2[0][0:64, sq], qp[0:64, 0:512],
                                     mybir.ActivationFunctionType.Identity,
                                     bias=bq01[0:64, :])
                nc.scalar.activation(qT2[1][0:64, sq], qp[64:128, 0:512],
                                     mybir.ActivationFunctionType.Identity,
                                     bias=bq01[64:128, :])
                nc.scalar.activation(qT2[2][0:64, sq], qp[0:64, 512:1024],
                                     mybir.ActivationFunctionType.Identity,
                                     bias=bq2[:])
                for h in range(HPC):
                    nc.vector.tensor_copy(out=qT2[h][64:128, sq], in_=qT2[h][0:64, sq])

            pending_norm = [None]

            def flush_norm():
                if pending_norm[0] is not None:
                    pending_norm[0]()
                    pending_norm[0] = None

            for c in range(KC // 2):
                qproj_step(0, c)
            qproj_drain(0)

            prev = [None]  # (ctx_mm, finish) of the previous head

            def finish_head(qc, h, ctx):
                # denominator row -> SBUF (on ACT), then gpsimd broadcasts
                # partition 0 to 64 partitions (the gpsimd engine is idle and
                # this avoids the old DRAM round trip); reciprocal + scale
                # are deferred further (flush_norm) so the broadcast latency
                # never blocks the DVE queue.
                sq = bass.ts(qc, 512)
                den_row = norm.tile([1, 512], F32, tag="den_row")
                nc.scalar.copy(den_row[:], ctx[64:65, :])
                di = qc * HPC + h
                nc.sync.dma_start(den_d[di : di + 1, :], den_row[:])
                den = norm.tile([64, 512], F32, tag="den")
                dsrc = den_d[di : di + 1, :]
                den_bcast = bass.AP(
                    tensor=dsrc.tensor,
                    offset=dsrc.offset,
                    ap=[[0, 64]] + list(dsrc.ap[1:]),
                )
                nc.sync.dma_start(den[:], den_bcast)

                def normalize():
                    rec = norm.tile([64, 512], F32, tag="rec")
                    nc.vector.reciprocal_approx_fast(out=rec[:], in_=den[:])
                    if h == 0:
                        nc.vector.tensor_mul(ctx01[0:64, sq], ctx[0:64, :], rec[:])
                    elif h == 1:
                        nc.vector.tensor_mul(ctx01[64:128, sq], ctx[0:64, :], rec[:])
                    else:
                        nc.vector.tensor_mul(ctx2[:, sq], ctx[0:64, :], rec[:])

                pending_norm[0] = normalize

            for qc in range(NSB):
                sq = bass.ts(qc, 512)
                for h in range(HPC):
                    ctx_t = bigp.tile([128, 512], F32, tag="big")
                    ctx = ctx_t[0:65, :]
                    pts = {}

                    def ctx_mm(g, ctx=ctx, h=h, pts=pts):
                        nc.tensor.matmul(
                            ctx,
                            v_dr[h][:, 2 * g : 2 * g + 2, 0:65],
                            pts.pop(g)[:].rearrange("p (a b) -> p a b", a=2),
                            start=(g == 0), stop=(g == 15),
                            perf_mode=DR,
                        )

                    for g in range(16):  # kv-chunk pairs
                        # previous head's last P@V rides in slot 0
                        if g == 0 and prev[0] is not None:
                            prev[0][0](15)
                            prev[0][1]()
                            prev[0] = None
                        if g >= 3:
                            ctx_mm(g - 3)
                        pt = work.tile([128, 1024], F8, tag="pt", bufs=8,
                                       name=f"pt_{qc}_{h}_{g}")
                        pts[g] = pt
                        sT = sp.tile([128, 1024], F32, tag="sT")
                        for j in (0, 1):
                            c = 2 * g + j
                            lo = 64 * j
                            nc.tensor.matmul(
                                sT[:, bass.ts(j, 512)],
                                kT2[h][lo : lo + 64, bass.ts(c, 128)],
                                qT2[h][lo : lo + 64, sq],
                            )
                        if g in DVE_PAIRS:
                            nc.vector.tensor_scalar(
                                pt.bitcast(U8)[:], sT[:], A_SCH, B_SCH,
                                mybir.AluOpType.mult, mybir.AluOpType.add,
                            )
                        else:
                            nc.scalar.activation(
                                pt[:], sT[:], mybir.ActivationFunctionType.Exp,
                                scale=SCALE,
                            )
                        if g == 6:
                            flush_norm()
                        # previous q-chunk's output projection, spread across
                        # heads 0 and 2
                        if h in (0, 2) and qc > 0 and g in (8, 11, 14):
                            op_chain(qc - 1, 3 * (h // 2) + (g - 8) // 3)
                        # next q-chunk's projection inside head 1's loop so
                        # its DVE drain lands well before head 0 of the next
                        # q chunk needs qT2
                        if h == 1 and qc + 1 < NSB:
                            if 1 <= g <= 3:
                                qproj_step(qc + 1, g - 1)
                            elif g == 4:
                                qproj_drain(qc + 1)
                    for gg in (13, 14):
                        ctx_mm(gg)
                    prev[0] = (ctx_mm, lambda qc=qc, h=h, ctx=ctx: finish_head(qc, h, ctx))
            # drain the last head
            ctx_mm_f, fin = prev[0]
            ctx_mm_f(15)
            fin()
            flush_norm()
            # last q-chunk's output projection (op tiles from the score
            # ring, which is idle by now)
            for d in range(KC):
                op_chain(NSB - 1, d, pool=sp)


_NC_CACHE = {}


def _build():
    if "nc" not in _NC_CACHE:
        nc = bacc.Bacc(
            "TRN2", target_bir_lowering=False, debug=False, num_devices=NC_CORES
        )
        with tile.TileContext(nc) as tc:
            _emit(tc)
        nc.compile()
        _NC_CACHE["nc"] = nc
    return _NC_CACHE["nc"]


def _f8(x):
    import ml_dtypes

    return x.astype(ml_dtypes.float8_e4m3fn)


def _tile_xT(x, dtype=np.float16):
    # x: [S, DM] fp32 -> x.T tiled as [NSB, 128, KC*512]: one contiguous
    # block per 512-seq chunk.
    xT = np.ascontiguousarray(x.T)  # [DM, S]
    t = np.ascontiguousarray(xT.reshape(KC, 128, NSB, 512).transpose(2, 1, 0, 3))
    if dtype is np.float16:
        return t.astype(np.float16)
    return _f8(t)


def make_in_maps(query, key, value, wq, bq, wk, bk, wv, bv, wo, bo):
    query = np.asarray(query)
    key = np.asarray(key)
    value = np.asarray(value)
    wq, bq, wk, bk, wv, bv, wo, bo = (
        np.asarray(a) for a in (wq, bq, wk, bk, wv, bv, wo, bo)
    )
    in_maps = []
    for c in range(NC_CORES):
        b = c // 4
        hs = (c % 4) * HPC * DK
        he = hs + HPC * DK
        in_maps.append(
            {
                "qTx": _tile_xT(query[b], dtype="f8"),
                "kTx": _tile_xT(key[b], dtype="f8"),
                "vTx": _tile_xT(value[b], dtype="f8"),
                "wqT": _f8(np.ascontiguousarray(wq[hs:he, :].T)),
                "wkT": _f8(np.ascontiguousarray(wk[hs:he, :].T)),
                "wvT": _f8(np.ascontiguousarray(wv[hs:he, :].T)),
                "woT": np.ascontiguousarray(wo[:, hs:he].T).astype(np.float16),
                "bq": bq[hs:he].reshape(-1, 1).astype(np.float32),
                "bk": bk[hs:he].reshape(-1, 1).astype(np.float32),
                "bv": bv[hs:he].reshape(-1, 1).astype(np.float32),
            }
        )
    return in_maps


def combine_outputs(results, bo):
    parts = [results[c]["out_pT"].astype(np.float32) for c in range(NC_CORES)]
    out0 = (parts[0] + parts[1] + parts[2] + parts[3]).T
    out1 = (parts[4] + parts[5] + parts[6] + parts[7]).T
    out = np.stack([out0, out1]) + np.asarray(bo)[None, None, :]
    return out.astype(np.float32)


def run_on_hw(in_maps, **kw):
    nc = _build()
    return run_bass_kernel_spmd(nc, in_maps, list(range(NC_CORES)), **kw)


def kernel(query, key, value, wq, bq, wk, bk, wv, bv, wo, bo):
    in_maps = make_in_maps(query, key, value, wq, bq, wk, bk, wv, bv, wo, bo)
    res = run_on_hw(in_maps)
    return combine_outputs(res.results, bo)

